# revision 11
# baseline (speedup 1.0000x reference)
"""Self-contained Trainium2 Bass kernel for nn_GCNResnet (batch-attention GCN).

Math (reference collapse):
  out = sum_n c_n * (softmax(X_n X_n^T) @ (X_n @ W)) / 1 + offset*(1_D @ W)
where X_n = x[:, n, :] ([B=4096, D=10]), c_n / offset fold BN(eval) + adjacency
+ avgpool scalars.  Per node the attention normalizer is folded into the PV
matmul via a ones column: U_n = E_n @ [c_n*(X_n@W) | 1], out_n = U[:, :10]/U[:, 10].

Sharding: row-slab parallel over 8 cores, 512 query rows per core. Each core
computes S^T chunks [128 keys x 512 rows] on PE (fp32r full-rate), exp on ACT
(no max subtraction; scores bounded ~45 << 88), PV accumulation in PSUM, small
PE transposes + DVE normalize for the tail. No collectives.
"""

import sys

if "/opt/trn_rl_repo" not in sys.path:
    sys.path.insert(0, "/opt/trn_rl_repo")

import numpy as np

import concourse.bass as bass
import concourse.mybir as mybir
from concourse import tile
from concourse.bass_utils import run_bass_kernel_spmd
from concourse.vector_clock import ScopedClock

B, N, D = 4096, 3, 10
NCORES = 8
R = B // NCORES            # 512 query rows per core
KC = B // 128              # 32 key chunks of 128
GROUP = 3                  # key chunks per psum buffer / exp call (row-tile width)
BN_EPS = 1e-5

# perf/numerics switches (exercised by test.py)
USE_F32R_SCORES = True
USE_F32R_PV = True
ROW_TILE = True


def _patched_drain_and_barrier(self, tick_clock, wait_clock):
    # Walrus in this container rejects >1 sync-wait on a CTRL-class
    # instruction; absorb the tail-drain waits into SP nops, one wait each.
    nc = self.nc
    probe = nc.sync.nop()
    wait_clock.add_sem_waits(probe.ins, ScopedClock({None: tick_clock.global_clock}))
    si = probe.ins.sync_info
    waits = list(si.on_wait) if si is not None else []
    upds = list(si.on_update) if si is not None else []
    probe.ins.sync_info = mybir.SyncInfo(on_wait=waits[:1], on_update=upds)
    for w in waits[1:]:
        n = nc.sync.nop()
        n.ins.sync_info = mybir.SyncInfo(on_wait=[w], on_update=[])
    nc.sync.drain()
    nc.all_engine_barrier()
    assert self.sems is not None
    popped = nc._tile_sem_poison_stack.pop()
    assert popped is self._sem_poison
    nc.clear_and_free_semaphores(list(self.sems.allocated().values()))
    nc.all_engine_barrier()


tile.TileContext._drain_and_barrier = _patched_drain_and_barrier

_MAX_WAITS = 1
_waitsplit_ctr = [0]


def _split_sync_waits(nc):
    """Walrus here allows very few sync-waits per instruction. Move excess
    waits onto same-engine no-ops placed immediately before the instruction
    (engine streams are in-order, so semantics are preserved)."""
    for f in nc.m.functions:
        for bb in f.blocks:
            new = []
            changed = False
            for inst in bb.instructions:
                si = inst.sync_info
                waits = list(si.on_wait) if si is not None else []
                if len(waits) > _MAX_WAITS:
                    changed = True
                    for w in waits[:-_MAX_WAITS]:
                        _waitsplit_ctr[0] += 1
                        nop = mybir.InstNoOp(
                            name=f"I-waitsplit-{_waitsplit_ctr[0]}", ins=[], outs=[]
                        )
                        nop.engine = inst.engine
                        nop.sync_info = mybir.SyncInfo(on_wait=[w], on_update=[])
                        new.append(nop)
                    inst.sync_info = mybir.SyncInfo(
                        on_wait=waits[-_MAX_WAITS:], on_update=list(si.on_update)
                    )
                new.append(inst)
            if changed:
                bb.instructions = new


def _mdt(use_f32r):
    # float32r is reduced-mantissa fp32 (TF32-like): matmul runs at full rate
    # (1 cycle/row vs 4 for fp32, moving dim >= 256) at ~1.8e-4 operand
    # rounding. Producer instructions must write fp32r (gpsimd casting DMA /
    # ACT output dtype); bitcasting raw fp32 is rejected by the BIR verifier.
    return mybir.dt.float32r if use_f32r else mybir.dt.float32


def build_nc(rep: int = 1) -> bass.Bass:
    """One-core SPMD program: full keys replicated, this core's 512-row slab."""
    f32 = mybir.dt.float32
    nc = bass.Bass()

    xt = nc.declare_dram_parameter("xt", [N, D, B], f32, isOutput=False)
    xts = nc.declare_dram_parameter("xts", [N, D, R], f32, isOutput=False)
    xh = nc.declare_dram_parameter("xh", [N, KC, 128, D + 1], f32, isOutput=False)
    ident = nc.declare_dram_parameter("ident", [128, D + 1], f32, isOutput=False)
    out = nc.declare_dram_parameter("out", [R, D], f32, isOutput=True)

    n_rt = 4 if ROW_TILE else 1  # partition replicas of xt for PE row tiling
    groups = []
    c = 0
    while c < KC:
        groups.append(list(range(c, min(c + GROUP, KC))))
        c += GROUP

    with tile.TileContext(nc) as tc:
        with (
            tc.tile_pool(name="xtp", bufs=1) as xtp,
            tc.tile_pool(name="xhp", bufs=1) as xhp,
            tc.tile_pool(name="cst", bufs=1) as cst,
            tc.tile_pool(name="etp", bufs=3) as etp,
            tc.tile_pool(name="tail", bufs=1) as tailp,
            tc.tile_pool(name="pss", bufs=2, space="PSUM") as pss,
            tc.tile_pool(name="psu", bufs=1, space="PSUM") as psu,
        ):
            for _ in range(rep):
                # ---- input loads ----
                # columns 0..B-1: all keys; columns B..B+R-1: this core's
                # query slab (replicated per row-tile partition offset)
                sdt = _mdt(USE_F32R_SCORES)
                pdt = _mdt(USE_F32R_PV)
                xt_sb = [
                    xtp.tile([128, B + R], sdt, tag=f"xt{n}", name=f"xt{n}")
                    for n in range(N)
                ]
                for n in range(N):
                    for i in range(n_rt):
                        nc.gpsimd.dma_start(xt_sb[n][32 * i : 32 * i + D, 0:B], xt[n])
                        nc.gpsimd.dma_start(
                            xt_sb[n][32 * i : 32 * i + D, B : B + R], xts[n]
                        )
                xh_sb = xhp.tile([128, N * KC * (D + 1)], pdt)
                xh_r = xh_sb[:].rearrange("p (n c d) -> p n c d", n=N, c=KC)
                for n in range(N):
                    nc.gpsimd.dma_start(
                        xh_r[:, n], xh[n].rearrange("c p d -> p c d")
                    )
                id_sb = cst.tile([128, D + 1], f32)
                nc.sync.dma_start(id_sb[:], ident[:])

                u_sb = tailp.tile([128, 512 * N], f32, tag="usb")
                for n in range(N):
                    u_ps = psu.tile([128, 512], f32, tag="u")
                    for g in groups:
                        w = len(g)
                        ps = pss.tile([128, 512 * GROUP], f32, tag="scores")
                        for i, ck in enumerate(g):
                            po = 32 * i if ROW_TILE else 0
                            nc.tensor.matmul(
                                ps[:, 512 * i : 512 * (i + 1)],
                                lhsT=xt_sb[n][po : po + D, 128 * ck : 128 * (ck + 1)],
                                rhs=xt_sb[n][po : po + D, B : B + R],
                                tile_position=(po, 0),
                            )
                        et = etp.tile([128, 512 * GROUP], pdt, tag="et")
                        nc.scalar.activation(
                            et[:, : 512 * w],
                            ps[:, : 512 * w],
                            mybir.ActivationFunctionType.Exp,
                        )
                        for i, ck in enumerate(g):
                            nc.tensor.matmul(
                                u_ps[0 : D + 1, :],
                                lhsT=xh_sb[
                                    :,
                                    (n * KC + ck) * (D + 1) : (n * KC + ck + 1) * (D + 1),
                                ],
                                rhs=et[:, 512 * i : 512 * (i + 1)],
                                start=(ck == 0),
                                stop=(ck == KC - 1),
                            )
                    # drain this node's accumulator so the bank can be reused
                    nc.vector.tensor_copy(
                        u_sb[0 : D + 1, 512 * n : 512 * (n + 1)], u_ps[0 : D + 1, :]
                    )

                # ---- tail: normalize + transpose to [rows, 10] ----
                vt_ps = psu.tile([128, N * 4 * (D + 1)], f32, tag="vt")
                for n in range(N):
                    for j in range(4):
                        nc.tensor.transpose(
                            vt_ps[:, (n * 4 + j) * (D + 1) : (n * 4 + j + 1) * (D + 1)],
                            u_sb[0 : D + 1, 512 * n + 128 * j : 512 * n + 128 * (j + 1)],
                            id_sb[0 : D + 1, :],
                        )
                out_sb = tailp.tile([128, 4 * D], f32, tag="osb")
                rec_sb = tailp.tile([128, 4 * N], f32, tag="rec")
                tmp_sb = tailp.tile([128, D], f32, tag="tmp")
                vt_r = vt_ps[:].rearrange("p (n j d) -> p n j d", n=N, j=4)
                for j in range(4):
                    nc.vector.reciprocal(
                        rec_sb[:, j * N : (j + 1) * N], vt_r[:, :, j, D]
                    )
                    for n in range(N):
                        dst = out_sb[:, j * D : (j + 1) * D] if n == 0 else tmp_sb[:]
                        nc.vector.tensor_scalar_mul(
                            dst,
                            vt_r[:, n, j, :D],
                            rec_sb[:, j * N + n : j * N + n + 1],
                        )
                        if n > 0:
                            nc.vector.tensor_add(
                                out_sb[:, j * D : (j + 1) * D],
                                out_sb[:, j * D : (j + 1) * D],
                                tmp_sb[:],
                            )
                    nc.sync.dma_start(
                        out[128 * j : 128 * (j + 1), :], out_sb[:, j * D : (j + 1) * D]
                    )
    _split_sync_waits(nc)
    return nc


def _host_prep(x, A, gc_weight, bn_gamma, bn_beta, bn_mean, bn_var):
    x = np.asarray(x, np.float32)
    A = np.asarray(A, np.float32)
    W = np.asarray(gc_weight, np.float32)
    scale = np.asarray(bn_gamma, np.float32) / np.sqrt(
        np.asarray(bn_var, np.float32) + BN_EPS
    )
    d_half = 0.5 * np.eye(N, dtype=np.float32)
    a0 = np.ones((N, N), np.float32) - np.eye(N, dtype=np.float32)
    adj = d_half @ (a0 + A) @ d_half
    wk = 0.5 * (adj[0] + adj[1])                      # [N]
    cn = (wk * scale).astype(np.float32)              # [N]
    offset = float(
        np.sum(wk * (np.asarray(bn_beta, np.float32)
                     - np.asarray(bn_mean, np.float32) * scale))
    )
    bias_vec = (offset * W.sum(axis=0)).astype(np.float32)  # [D]

    xt = np.ascontiguousarray(x.transpose(1, 2, 0))   # [N, D, B]
    xh = np.empty((N, B, D + 1), np.float32)
    for n in range(N):
        xh[n, :, :D] = (x[:, n, :] @ W) * cn[n]
        xh[n, :, D] = 1.0
    xh = np.ascontiguousarray(xh.reshape(N, KC, 128, D + 1))
    ident = np.zeros((128, D + 1), np.float32)
    for n in range(N):
        ident[32 * n : 32 * n + D + 1, :] = np.eye(D + 1, dtype=np.float32)
    return xt, xh, ident, bias_vec


def _in_maps(xt, xh, ident):
    maps = []
    for c in range(NCORES):
        sl = np.ascontiguousarray(xt[:, :, c * R : (c + 1) * R])  # [N, D, R]
        maps.append({"xt": xt, "xts": sl, "xh": xh, "ident": ident})
    return maps


def kernel(**inputs) -> np.ndarray:
    assert inputs["x"].shape == (B, N, D)
    xt, xh, ident, bias_vec = _host_prep(**inputs)
    nc = build_nc(rep=1)
    res = run_bass_kernel_spmd(nc, _in_maps(xt, xh, ident), list(range(NCORES)))
    out = np.concatenate([res.results[c]["out"] for c in range(NCORES)], axis=0)
    return (out + bias_vec[None, :]).astype(np.float32)
